# revision 14
# baseline (speedup 1.0000x reference)
"""GBST layer (pooling) Trainium2 Bass/Tile kernel — pipelined version.

Math (per sample, x [512, 8192]):
  y = conv1d(x, W[512,512,5], b, VALID)                    # [512, 8188]
  r[l] = score . y[:, l]                                   # conv'd scores
  For w in {1,2,3}: cand_w = block-mean(y, w); s_w = block-mean(r, w)
  att = softmax over the 3 per-position scores; out[l] = sum_w att_w[l] * cand_w(l)
  out_ds = pairwise mean of out                            # [512, 4096]

Strategy: 1 sample per NeuronCore (8 cores, batch-parallel).
  - conv as 5 shifted bf16 matmuls per (oc, ic) chunk pair on PE (1280 MMs)
  - y kept fully resident in SBUF as bf16 [4][128, 8208] (zero tail
    reproduces the reference's zero-pad semantics exactly)
  - r via PE (score^T @ y), written to DRAM, re-read in an "L6" layout
    [128, 66] where partition j holds positions 768c+6j+u
  - softmax + downsample folded into 4 coefficient rows A,B,F,G:
      out_ds[:,p] = A[p]*y[:,2p] + B[p]*y[:,2p+1]
                  + F[p]*S3[:,g0(p)] + (p%3==1)*G[p]*S3[:,g1(p)]
    with S3 = stride-3 running sums of y, g0=floor(2p/3), g1=floor((2p+1)/3).
    For p%3 in {0,2}, g0==g1, so the two w=3 terms merge into F.
  - PIPELINED: coefficient blocks and combine tiles are emitted between
    conv tiles as soon as their inputs exist, so DVE/Pool combine work
    overlaps PE conv work instead of running after it.
  - combine work split DVE (A/B part + v==1 terms) / Pool (S3 build +
    v in {0,2} terms) to use both engines.

This walrus build caps semaphore waits per instruction very low, so
_fix_wait_overflow() hoists excess waits onto injected same-engine NOPs.
"""

import numpy as np
import ml_dtypes

import concourse.bass as bass
import concourse.mybir as mybir
from concourse.tile import TileContext

BF16 = mybir.dt.bfloat16
F32 = mybir.dt.float32
AF = mybir.ActivationFunctionType
ALU = mybir.AluOpType

N_CORES = 8
E, L, KS = 512, 8192, 5
LC = L - KS + 1          # 8188 valid conv outputs
LPAD = 8208              # y cols incl. zero tail (S3 reads up to col 8192)
NCB = 11                 # L6-layout column blocks: 768*11 = 8448 >= L
RPAD = 768 * NCB         # padded r length
PCO = 384 * NCB          # coeff row length (>= 4096)
NT = 16                  # conv tiles of 512 positions
OUTL = L // 2            # 4096
NS3T = 688               # max S3 cols materialized per combine tile

REP = 1                  # whole-body repetitions (timing builds only)

_BUILT = None


def _sap(tile_ap, col_off, dims):
    """Strided SBUF AP on a pool tile: partition dim + custom free dims."""
    pitch, nparts = tile_ap.ap[0]
    return bass.AP(tile_ap.tensor, tile_ap.offset + col_off, [[pitch, nparts]] + dims)


def _fix_wait_overflow(nc):
    """Split >limit semaphore waits onto injected same-engine NOPs."""
    cnt = 0
    for f in nc.m.functions:
        for b in f.blocks:
            newlist = []
            for inst in b.instructions:
                si = inst.sync_info
                if si is not None and si.on_wait:
                    lim = 1
                    waits = list(si.on_wait)
                    while len(waits) > lim:
                        w = waits.pop(0)
                        nop = mybir.InstNoOp(name=f"wfx-{cnt}")
                        cnt += 1
                        nop.engine = inst.engine
                        nop.sync_info = mybir.SyncInfo(on_wait=[w], on_update=[])
                        newlist.append(nop)
                    if cnt and len(waits) != len(si.on_wait):
                        inst.sync_info = mybir.SyncInfo(
                            on_wait=waits, on_update=list(si.on_update)
                        )
                newlist.append(inst)
            b.instructions[:] = newlist
    return cnt


def _build_bass(fix_waits=True, rep=None):
    if rep is None:
        rep = REP
    nc = bass.Bass("TRN2", target_bir_lowering=False, num_devices=N_CORES)

    xb = nc.dram_tensor("xb", [E, L], BF16, kind="ExternalInput")
    wsb = nc.dram_tensor("wsb", [128, KS * 4 * 4 * 128], BF16, kind="ExternalInput")
    scs = nc.dram_tensor("scs", [128, 4], BF16, kind="ExternalInput")
    bis = nc.dram_tensor("bis", [128, 4], F32, kind="ExternalInput")
    out_d = nc.dram_tensor("out", [E, OUTL], F32, kind="ExternalOutput")
    coef_d = nc.dram_tensor("coef", [4, PCO], F32)
    r_d = nc.dram_tensor("r_scratch", [RPAD], F32)

    with TileContext(nc) as tc:
        with (
            tc.tile_pool(name="const", bufs=1) as kpool,
            tc.tile_pool(name="ybuf", bufs=1) as ypool,
            tc.tile_pool(name="xin", bufs=4) as xpool,
            tc.tile_pool(name="ps", bufs=4, space="PSUM") as pspool,
            tc.tile_pool(name="psr", bufs=2, space="PSUM") as prpool,
            tc.tile_pool(name="sc", bufs=1) as spool,
            tc.tile_pool(name="cf", bufs=2) as cpool,
            tc.tile_pool(name="s3", bufs=3) as s3pool,
            tc.tile_pool(name="otp", bufs=3) as otpool,
            tc.tile_pool(name="tbp", bufs=2) as tbpool,
            tc.tile_pool(name="tdp", bufs=4) as tdpool,
        ):
            # const tiles; DMA emission order is interleaved with the first
            # body's x prefetch (see _emit_body) so only w(oc0)+x(tile0)
            # gate the first conv matmul
            w_sb = kpool.tile([128, KS * 4 * 4 * 128], BF16, tag="w")
            sc_sb = kpool.tile([128, 4], BF16, tag="sc")
            bi_sb = kpool.tile([128, 4], F32, tag="bi")

            def load_consts():
                # w oc-chunks 1-3 loaded later (not needed immediately)
                nc.sync.dma_start(out=bi_sb[:], in_=bis[:])
                nc.sync.dma_start(out=sc_sb[:], in_=scs[:])
                for oc in range(1, 4):
                    nc.sync.dma_start(
                        out=w_sb[:, oc * 2560 : (oc + 1) * 2560],
                        in_=wsb[:, oc * 2560 : (oc + 1) * 2560],
                    )

            def load_w0():
                nc.sync.dma_start(out=w_sb[:, 0:2560], in_=wsb[:, 0:2560])

            consts = (load_w0, load_consts)
            for _ in range(rep):
                _emit_body(nc, tc, xb, out_d, coef_d, r_d, w_sb, sc_sb, bi_sb,
                           ypool, xpool, pspool, prpool, spool, cpool,
                           s3pool, otpool, tbpool, tdpool, consts)
                consts = None

    if fix_waits:
        _fix_wait_overflow(nc)
    return nc


def _emit_body(nc, tc, xb, out_d, coef_d, r_d, w_sb, sc_sb, bi_sb,
               ypool, xpool, pspool, prpool, spool, cpool,
               s3pool, otpool, tbpool, tdpool, consts=None):
    ys = [
        ypool.tile([128, LPAD], BF16, name=f"y{c}", tag=f"y{c}")
        for c in range(4)
    ]

    # persistent phase-2 tiles (written in column slices per group)
    r6 = spool.tile([128, 66], F32, tag="r6")
    e1 = spool.tile([128, 66], F32, tag="e1")
    s2h = spool.tile([128, 33], F32, tag="s2h")
    e2 = spool.tile([128, 33], F32, tag="e2")
    s3h = spool.tile([128, 22], F32, tag="s3h")
    e3 = spool.tile([128, 22], F32, tag="e3")
    den = spool.tile([128, 66], F32, tag="den")
    rec = spool.tile([128, 66], F32, tag="rec")
    t1 = spool.tile([128, 66], F32, tag="t1")
    recsum = spool.tile([128, 33], F32, tag="recsum")
    # coefs sections: A at 0, B at 33, F at 66, G at 99; col 33*i + 3c + v
    coefs = spool.tile([128, 132], F32, tag="coefs")

    # ---- conv tile: x prefetch (SP DMA) separated from the matmuls so the
    # prefetch for tile t+1 is queued before emit_r(t-1)'s waiting DMA ----
    xts = {}

    def prefetch_x(t):
        n0 = 512 * t
        xw = min(516, L - n0)
        xt = xpool.tile([128, 4 * 516], BF16, tag="xt")
        # one packed DMA: partition p, chunk ic, col j
        nc.sync.dma_start(
            out=_sap(xt, 0, [[516, 4], [1, xw]]),
            in_=bass.AP(xb, n0, [[L, 128], [128 * L, 4], [1, xw]]),
        )
        xts[t] = xt

    def conv_tile(t):
        n0 = 512 * t
        n = min(512, LC - n0)
        xt = xts.pop(t)
        for oc in range(4):
            py = pspool.tile([128, 512], F32, tag="py")
            first = True
            for ic in range(4):
                for k in range(KS):
                    woff = (oc * KS * 4 + k * 4 + ic) * 128
                    nc.tensor.matmul(
                        py[:, :n],
                        lhsT=w_sb[:, woff : woff + 128],
                        rhs=xt[:, ic * 516 + k : ic * 516 + k + n],
                        start=first,
                        stop=(ic == 3 and k == KS - 1),
                    )
                    first = False
            nc.scalar.activation(
                ys[oc][:, n0 : n0 + n], py[:, :n], AF.Identity,
                bias=bi_sb[:, oc : oc + 1], scale=1.0,
            )

    # ---- scores tile: r[n0:n0+n] = score . y ----
    def emit_r(t):
        n0 = 512 * t
        n = min(512, LC - n0)
        pr = prpool.tile([1, 512], F32, tag="pr")
        for cc in range(4):
            nc.tensor.matmul(
                pr[:, :n],
                lhsT=sc_sb[:, cc : cc + 1],
                rhs=ys[cc][:, n0 : n0 + n],
                start=(cc == 0),
                stop=(cc == 3),
            )
        rsb = xpool.tile([1, 512], F32, tag="rsb")
        nc.scalar.activation(rsb[:1, :n], pr[:1, :n], AF.Copy)
        nc.sync.dma_start(out=bass.AP(r_d, n0, [[1, n]]), in_=rsb[:1, :n])

    # ---- phase 2 for c-blocks [cl, ch]: coefficient rows ----
    def phase2(cl, ch):
        nb = ch - cl + 1
        # r6[j, 6c+u] = r[768c + 6j + u], one packed DMA
        nc.sync.dma_start(
            out=_sap(r6, 6 * cl, [[6, nb], [1, 6]]),
            in_=bass.AP(r_d, 768 * cl, [[6, 128], [768, nb], [1, 6]]),
        )
        nc.scalar.activation(e1[:, 6 * cl : 6 * cl + 6 * nb],
                             r6[:, 6 * cl : 6 * cl + 6 * nb], AF.Exp)
        # s2h[j,3c+v] = r6[,6c+2v] + r6[,6c+2v+1];  e2 = exp(s2h/2)
        nc.vector.tensor_add(
            out=_sap(s2h, 3 * cl, [[3, nb], [1, 3]]),
            in0=_sap(r6, 6 * cl, [[6, nb], [2, 3]]),
            in1=_sap(r6, 6 * cl + 1, [[6, nb], [2, 3]]),
        )
        nc.scalar.activation(e2[:, 3 * cl : 3 * cl + 3 * nb],
                             s2h[:, 3 * cl : 3 * cl + 3 * nb], AF.Exp, scale=0.5)
        # s3h[j,2c+w] = sum of r6[,6c+3w+{0,1,2}];  e3 = exp(s3h/3)
        nc.vector.tensor_add(
            out=_sap(s3h, 2 * cl, [[2, nb], [1, 2]]),
            in0=_sap(r6, 6 * cl, [[6, nb], [3, 2]]),
            in1=_sap(r6, 6 * cl + 1, [[6, nb], [3, 2]]),
        )
        nc.vector.tensor_add(
            out=_sap(s3h, 2 * cl, [[2, nb], [1, 2]]),
            in0=_sap(s3h, 2 * cl, [[2, nb], [1, 2]]),
            in1=_sap(r6, 6 * cl + 2, [[6, nb], [3, 2]]),
        )
        nc.scalar.activation(e3[:, 2 * cl : 2 * cl + 2 * nb],
                             s3h[:, 2 * cl : 2 * cl + 2 * nb], AF.Exp,
                             scale=1.0 / 3.0)
        # den = e1 + expand2(e2) + expand3(e3); rec = 1/den
        for v in range(3):
            nc.vector.tensor_add(
                out=_sap(den, 6 * cl + 2 * v, [[6, nb], [1, 2]]),
                in0=_sap(e1, 6 * cl + 2 * v, [[6, nb], [1, 2]]),
                in1=_sap(e2, 3 * cl + v, [[3, nb], [0, 2]]),
            )
        for w in range(2):
            nc.vector.tensor_add(
                out=_sap(den, 6 * cl + 3 * w, [[6, nb], [1, 3]]),
                in0=_sap(den, 6 * cl + 3 * w, [[6, nb], [1, 3]]),
                in1=_sap(e3, 2 * cl + w, [[2, nb], [0, 3]]),
            )
        nc.vector.reciprocal(rec[:, 6 * cl : 6 * cl + 6 * nb],
                             den[:, 6 * cl : 6 * cl + 6 * nb])
        # t1 = e1*rec ; recsum[3c+v] = rec[6c+2v]+rec[6c+2v+1]
        nc.vector.tensor_mul(out=t1[:, 6 * cl : 6 * cl + 6 * nb],
                             in0=e1[:, 6 * cl : 6 * cl + 6 * nb],
                             in1=rec[:, 6 * cl : 6 * cl + 6 * nb])
        nc.vector.tensor_add(
            out=_sap(recsum, 3 * cl, [[3, nb], [1, 3]]),
            in0=_sap(rec, 6 * cl, [[6, nb], [2, 3]]),
            in1=_sap(rec, 6 * cl + 1, [[6, nb], [2, 3]]),
        )
        # e2r = 0.25 * e2 * recsum -> reuse s2h as scratch
        nc.vector.scalar_tensor_tensor(
            out=_sap(s2h, 3 * cl, [[3, nb], [1, 3]]),
            in0=_sap(e2, 3 * cl, [[3, nb], [1, 3]]), scalar=0.25,
            in1=_sap(recsum, 3 * cl, [[3, nb], [1, 3]]),
            op0=ALU.mult, op1=ALU.mult,
        )
        # A = 0.5*t1[even] + e2r ; B = 0.5*t1[odd] + e2r
        nc.vector.scalar_tensor_tensor(
            out=_sap(coefs, 3 * cl, [[3, nb], [1, 3]]),
            in0=_sap(t1, 6 * cl, [[6, nb], [2, 3]]),
            scalar=0.5, in1=_sap(s2h, 3 * cl, [[3, nb], [1, 3]]),
            op0=ALU.mult, op1=ALU.add,
        )
        nc.vector.scalar_tensor_tensor(
            out=_sap(coefs, 33 + 3 * cl, [[3, nb], [1, 3]]),
            in0=_sap(t1, 6 * cl + 1, [[6, nb], [2, 3]]),
            scalar=0.5, in1=_sap(s2h, 3 * cl, [[3, nb], [1, 3]]),
            op0=ALU.mult, op1=ALU.add,
        )
        # F row: v=0: e3[2c]*recsum[3c]/6 ; v=1: e3[2c]*rec[6c+2]/6 ;
        #        v=2: e3[2c+1]*recsum[3c+2]/6
        fargs = [(0, recsum, 3 * cl, 3), (0, rec, 6 * cl + 2, 6),
                 (1, recsum, 3 * cl + 2, 3)]
        for v, (w0, src, soff, sstr) in enumerate(fargs):
            nc.vector.scalar_tensor_tensor(
                out=_sap(coefs, 66 + 3 * cl + v, [[3, nb]]),
                in0=_sap(e3, 2 * cl + w0, [[2, nb]]),
                scalar=1.0 / 6.0,
                in1=_sap(src, soff, [[sstr, nb]]),
                op0=ALU.mult, op1=ALU.mult,
            )
        # G row (only v=1 used): e3[2c+w1]*rec[6c+2v+1]/6, w1=(0,1,1)
        for v, w1 in enumerate([0, 1, 1]):
            nc.vector.scalar_tensor_tensor(
                out=_sap(coefs, 99 + 3 * cl + v, [[3, nb]]),
                in0=_sap(e3, 2 * cl + w1, [[2, nb]]),
                scalar=1.0 / 6.0,
                in1=_sap(rec, 6 * cl + 2 * v + 1, [[6, nb]]),
                op0=ALU.mult, op1=ALU.mult,
            )
        # reorder DMAs: p = 384c + 3j + v, one per coefficient row
        for i in range(4):
            nc.sync.dma_start(
                out=bass.AP(coef_d, i * PCO + 384 * cl,
                            [[3, 128], [384, nb], [1, 3]]),
                in_=_sap(coefs, 33 * i + 3 * cl, [[3, nb], [1, 3]]),
            )

    # ---- combine: one cc-chunk of output tile [p0, p0+width) ----
    def combine_cofs(p0, width):
        cofs = cpool.tile([128, 4096], F32, tag="cofs")
        nc.sync.dma_start(
            out=_sap(cofs, 0, [[1024, 4], [1, width]]),
            in_=bass.AP(coef_d, p0, [[0, 128], [PCO, 4], [1, width]]),
        )
        return cofs

    def combine_cc(p0, width, cc, cofs):
        ms = (2 * p0) // 3
        gmax = (2 * (p0 + width - 1) + 1) // 3
        nb3 = gmax - ms + 1
        yt = ys[cc]
        s3t = s3pool.tile([128, NS3T], F32, tag="s3t")
        nc.gpsimd.tensor_add(
            out=s3t[:, :nb3],
            in0=_sap(yt, 3 * ms, [[3, nb3]]),
            in1=_sap(yt, 3 * ms + 1, [[3, nb3]]),
        )
        nc.gpsimd.tensor_add(
            out=s3t[:, :nb3], in0=s3t[:, :nb3],
            in1=_sap(yt, 3 * ms + 2, [[3, nb3]]),
        )
        ot = otpool.tile([128, 1024], F32, tag="ot")
        tb = tbpool.tile([128, 1024], F32, tag="tb")
        nc.vector.tensor_mul(
            out=ot[:, :width], in0=_sap(yt, 2 * p0, [[2, width]]),
            in1=cofs[:, 0:width],
        )
        nc.vector.tensor_mul(
            out=tb[:, :width], in0=_sap(yt, 2 * p0 + 1, [[2, width]]),
            in1=cofs[:, 1024 : 1024 + width],
        )
        nc.vector.tensor_add(out=ot[:, :width], in0=ot[:, :width],
                             in1=tb[:, :width])
        for vg in range(3):       # vg = p mod 3
            ov = (vg - p0) % 3    # local offset within the tile
            nq = len(range(ov, width, 3))
            g0 = (2 * (p0 + ov)) // 3 - ms
            g1 = (2 * (p0 + ov) + 1) // 3 - ms
            if vg == 1:
                td = tdpool.tile([128, 342], F32, tag="td")
                te = tdpool.tile([128, 342], F32, tag="te")
                nc.vector.tensor_mul(
                    out=td[:, :nq], in0=_sap(s3t, g0, [[2, nq]]),
                    in1=_sap(cofs, 2048 + ov, [[3, nq]]),
                )
                nc.vector.tensor_mul(
                    out=te[:, :nq], in0=_sap(s3t, g1, [[2, nq]]),
                    in1=_sap(cofs, 3072 + ov, [[3, nq]]),
                )
                nc.vector.tensor_add(out=td[:, :nq], in0=td[:, :nq],
                                     in1=te[:, :nq])
                nc.vector.tensor_add(
                    out=_sap(ot, ov, [[3, nq]]),
                    in0=_sap(ot, ov, [[3, nq]]), in1=td[:, :nq],
                )
            else:
                td = tdpool.tile([128, 342], F32, tag="tp")
                nc.gpsimd.tensor_mul(
                    out=td[:, :nq], in0=_sap(s3t, g0, [[2, nq]]),
                    in1=_sap(cofs, 2048 + ov, [[3, nq]]),
                )
                nc.gpsimd.tensor_add(
                    out=_sap(ot, ov, [[3, nq]]),
                    in0=_sap(ot, ov, [[3, nq]]), in1=td[:, :nq],
                )
        nc.sync.dma_start(
            out=out_d[128 * cc : 128 * (cc + 1), p0 : p0 + width],
            in_=ot[:, :width],
        )

    # ---- pipelined emission schedule ----
    # phase2 groups: (cl, ch) gated by r tiles (emit_r(t-1) runs at step t)
    # combine tiles: (p0, width) gated by conv tiles + coef groups
    cofs_cache = {}

    def do_combine(p0, width, cc):
        if (p0, width) not in cofs_cache:
            cofs_cache[(p0, width)] = combine_cofs(p0, width)
        combine_cc(p0, width, cc, cofs_cache[(p0, width)])

    schedule = {
        5: [("p2", 0, 2), ("cb", 0, 1024, 0)],
        6: [("cb", 0, 1024, 1)],
        7: [("cb", 0, 1024, 2)],
        8: [("cb", 0, 1024, 3)],
        9: [("p2", 3, 5), ("cb", 1024, 1024, 0)],
        10: [("cb", 1024, 1024, 1), ("cb", 1024, 1024, 2)],
        11: [("cb", 1024, 1024, 3)],
        12: [("p2", 6, 7), ("cb", 2048, 1024, 0)],
        13: [("cb", 2048, 1024, 1), ("cb", 2048, 1024, 2)],
        14: [("cb", 2048, 1024, 3)],
        15: [("p2", 8, 9), ("cb", 3072, 768, 0), ("cb", 3072, 768, 1),
             ("cb", 3072, 768, 2), ("cb", 3072, 768, 3)],
    }

    if consts is not None:
        consts[0]()          # w oc0
    prefetch_x(0)
    if consts is not None:
        consts[1]()          # bias, score, w oc1-3
    for c in range(4):
        nc.gpsimd.memset(ys[c][:, LC:LPAD], 0.0)
    zr = spool.tile([1, RPAD - LC], F32, tag="zr")
    nc.gpsimd.memset(zr[:], 0.0)
    nc.sync.dma_start(out=bass.AP(r_d, LC, [[1, RPAD - LC]]), in_=zr[:1, :])
    for t in range(NT):
        if t + 1 < NT:
            prefetch_x(t + 1)
        # emit_r BEFORE conv tile t, so the 4 score matmuls for tile t-1
        # run ahead of tile t's 80 conv matmuls in PE program order
        if t >= 1:
            emit_r(t - 1)
        conv_tile(t)
        for item in schedule.get(t, []):
            if item[0] == "p2":
                phase2(item[1], item[2])
            else:
                do_combine(item[1], item[2], item[3])

    # tail: only the last 256 output cols truly need conv tile 15
    emit_r(15)
    phase2(10, 10)
    for cc in range(4):
        do_combine(3840, 256, cc)


def _prep_inputs(x, conv_w, conv_b, score_w):
    """Per-core input maps. Core b processes sample b."""
    bf = ml_dtypes.bfloat16
    wT = np.ascontiguousarray(conv_w.transpose(1, 0, 2))  # [in, out, k]
    wsb = np.empty((128, KS * 4 * 4 * 128), dtype=bf)
    for oc in range(4):
        for k in range(KS):
            for ic in range(4):
                off = (oc * KS * 4 + k * 4 + ic) * 128
                wsb[:, off : off + 128] = wT[
                    128 * ic : 128 * (ic + 1), 128 * oc : 128 * (oc + 1), k
                ].astype(bf)
    scs = np.ascontiguousarray(score_w.reshape(4, 128).T).astype(bf)
    bis = np.ascontiguousarray(conv_b.reshape(4, 128).T.astype(np.float32))
    maps = []
    for b in range(N_CORES):
        maps.append({"xb": x[b].astype(bf), "wsb": wsb, "scs": scs, "bis": bis})
    return maps


def kernel(x, conv_w, conv_b, score_w):
    global _BUILT
    from concourse.bass_utils import run_bass_kernel_spmd

    if _BUILT is None:
        _BUILT = _build_bass()
    nc = _BUILT
    x = np.asarray(x, dtype=np.float32)
    maps = _prep_inputs(
        x,
        np.asarray(conv_w, dtype=np.float32),
        np.asarray(conv_b, dtype=np.float32),
        np.asarray(score_w, dtype=np.float32),
    )
    res = run_bass_kernel_spmd(nc, maps, core_ids=list(range(N_CORES)))
    out = np.stack([r["out"] for r in res.results], axis=0)
    return out.astype(np.float32)
